# revision 1
# baseline (speedup 1.0000x reference)
"""Trainium2 Bass kernel v2 for single-head self-attention, x:[8,384,56,56].

Math per batch b (X = x[b] reshaped [C=384, N=3136]):
    Q = w1 @ X; V = w2 @ X; S^T = X^T Q (scaled)
    A^T = exp(S^T); O = V @ A^T / colsum(A^T)   (O in [C, N] layout)

Backend cost model (measured on this stack): ~40us flat per instruction
(GEMM math ~free), ACT ~1.2ns/elem, DVE copy ~2ns/elem, DVE strided
reduce ~0.6ns/elem, gpsimd all_reduce ~1.5ms/op. So: minimize
instruction count, avoid gpsimd entirely, prefer ACT for big PSUM->SBUF
copies, use tiny matmuls for cross-partition sums/broadcasts.

Design:
  - X padded to N_PAD=3200 (25 full 128-row m-tiles, no ragged tile).
    Pad columns are zero => fake S^T rows are 0 => exp gives exactly 1;
    the constant 64 is subtracted from the column sum, and V^T pad rows
    are exactly 0 so the PV product is unaffected.
  - softmax denominator: DVE reduce over m-tiles -> [128,w], ones-lhsT
    matmul column-sum -> psum [1,w], one tensor_scalar_add moves it to
    SBUF and drops the padding contribution, DVE reciprocal, K=1
    ones-matmul broadcast -> [128,w], ACT copy to SBUF for the final
    normalize multiply. (No gpsimd.)
  - PSUM->SBUF bulk copies on the scalar engine (ACT Copy).
  - max-subtraction skipped (logits ~N(0,1); exp is safe in f32).
  - A^T, V^T in f32: bf16 weights would trigger FWL => a separate
    Ldweights instruction per PV matmul (525 extra instructions ~ 20ms on
    this backend). f32 weights fold the load into Matmult. O is streamed
    to DRAM per chunk so the f32 A^T/V^T fit in SBUF.
"""

import sys

import numpy as np

sys.path.insert(0, "/opt/trn_rl_repo")

import concourse.bass as bass  # noqa: E402
import concourse.tile as tile  # noqa: E402
from concourse import bacc, mybir  # noqa: E402
from concourse.bass_utils import run_bass_kernel_spmd  # noqa: E402

F32 = mybir.dt.float32
F32R = mybir.dt.float32r
BF16 = mybir.dt.bfloat16
EXP = mybir.ActivationFunctionType.Exp
COPY = mybir.ActivationFunctionType.Copy

C = 384
N = 3136
NP = 3200  # padded so all m-tiles are 128 rows
MT = 25
CT = 3
SCALE = float(C) ** -0.5
CW = 448
CHUNKS = [(i * CW, CW) for i in range(7)]
N_CORES = 8


def build_bass(reps: int = 1):
    nc = bacc.Bacc("TRN2", target_bir_lowering=False, debug=False)
    xb = nc.dram_tensor("xb", [C, NP], F32R, kind="ExternalInput")
    wcat = nc.dram_tensor("wcat", [C, 2 * C], F32R, kind="ExternalInput")
    out = nc.dram_tensor("out", [C, N], F32, kind="ExternalOutput")

    with tile.TileContext(nc) as tc:
        with (
            tc.tile_pool(name="persist", bufs=1) as persist,
            tc.tile_pool(name="spool", bufs=1, space="PSUM") as spool,
            tc.tile_pool(name="opool", bufs=1, space="PSUM") as opool,
        ):
            X = persist.tile([128, CT, NP], F32R, tag="X")
            Q = persist.tile([128, CT, N], F32R, tag="Q")
            VT = persist.tile([128, MT, C], F32, tag="VT")
            AT = persist.tile([128, MT, CW], F32, tag="AT")
            W = persist.tile([128, CT, 2 * C], F32R, tag="W")
            OSB = persist.tile([128, CT, CW], F32, tag="OSB")
            # fat ones tile: sliced as [128,1] (column-sum lhsT) and [1,128]
            # (broadcast lhsT); a fat memset passes the ISA width checks
            ones = persist.tile([128, 128], F32, tag="ones")
            msum = persist.tile([128, CW], F32, tag="msum")
            rrow = persist.tile([1, CW], F32, tag="rrow")
            rinv = persist.tile([1, CW], F32, tag="rinv")
            rbc = persist.tile([128, CW], F32, tag="rbc")

            # X arrives host-padded to NP columns (pad zeros), so no
            # on-device memset of an f32r tile is needed (walrus rejects it)
            for ct in range(CT):
                r = slice(128 * ct, 128 * (ct + 1))
                nc.sync.dma_start(out=X[:, ct, :], in_=xb[r, :])
                nc.sync.dma_start(out=W[:, ct, :], in_=wcat[r, :])
            nc.vector.memset(ones[:, :], 1.0)

            for _rep in range(reps):
                _emit(nc, spool, opool, X, Q, VT, AT, W, OSB,
                      ones, msum, rrow, rinv, rbc, out)

    nc.compile()
    return nc


def _emit(nc, spool, opool, X, Q, VT, AT, W, OSB,
          ones, msum, rrow, rinv, rbc, out):
    # ---- Q = w1 @ X  (Q[p, dt, n], d = dt*128+p) ----
    for dt in range(CT):
        ds = slice(128 * dt, 128 * (dt + 1))
        for b0, nb in ((0, 5), (5, 2)):
            sp = spool.tile([128, 5, 512], F32, tag="s")
            for j in range(nb):
                n0 = (b0 + j) * CW
                for ct in range(CT):
                    nc.tensor.matmul(
                        sp[:, j, :CW],
                        lhsT=W[:, ct, ds],
                        rhs=X[:, ct, n0 : n0 + CW],
                        start=(ct == 0),
                        stop=(ct == CT - 1),
                    )
            qdst = Q[:, dt, b0 * CW : (b0 + nb) * CW].rearrange(
                "p (b w) -> p b w", w=CW
            )
            nc.scalar.activation(out=qdst, in_=sp[:, :nb, :CW], func=COPY)

    # ---- V^T = (w2 @ X)^T  (VT[p, mt, d], m = mt*128+p); pad rows are 0 ----
    for g in range(5):
        sp = spool.tile([128, 5, 512], F32, tag="s")
        for j in range(5):
            mt = 5 * g + j
            ms = slice(128 * mt, 128 * (mt + 1))
            for ct in range(CT):
                nc.tensor.matmul(
                    sp[:, j, :C],
                    lhsT=X[:, ct, ms],
                    rhs=W[:, ct, C : 2 * C],
                    start=(ct == 0),
                    stop=(ct == CT - 1),
                )
        nc.scalar.activation(
            out=VT[:, 5 * g : 5 * g + 5, :], in_=sp[:, :5, :C], func=COPY
        )

    # ---- main loop over n-chunks ----
    for n0, w in CHUNKS:
        ns = slice(n0, n0 + w)
        # S^T tiles + exp -> A^T (5 groups of 5 m-tiles in 5 psum banks)
        for g in range(5):
            sp = spool.tile([128, 5, 512], F32, tag="s")
            for j in range(5):
                mt = 5 * g + j
                ms = slice(128 * mt, 128 * (mt + 1))
                for dt in range(CT):
                    nc.tensor.matmul(
                        sp[:, j, :w],
                        lhsT=X[:, dt, ms],
                        rhs=Q[:, dt, ns],
                        start=(dt == 0),
                        stop=(dt == CT - 1),
                    )
            # padding rows of tile 24 have S=0 -> exp=1 exactly; their +64
            # on the column sum is removed below, and V^T pad rows are 0 so
            # the PV product is unaffected.
            nc.scalar.activation(
                out=AT[:, 5 * g : 5 * g + 5, :w],
                in_=sp[:, :5, :w],
                func=EXP,
                scale=SCALE,
            )

        # denominator: reduce over m-tiles, column-sum matmul, reciprocal,
        # K=1 broadcast matmul into psum (bank 0/1 of the S pool, now idle)
        atp = AT[:, :, :w].rearrange("p m w -> p w m")
        nc.vector.reduce_sum(msum[:, :w], atp, axis=mybir.AxisListType.X)
        aux = spool.tile([128, 5, 512], F32, tag="s")
        nc.tensor.matmul(
            aux[0:1, 0, :w], lhsT=ones[:, 0:1], rhs=msum[:, :w],
            start=True, stop=True,
        )
        # copy out of psum and remove the NP-N=64 padding rows' exp(0)=1
        # contribution in one DVE op (reads one psum operand - legal)
        nc.vector.tensor_scalar_add(out=rrow[:, :w], in0=aux[0:1, 0, :w],
                                    scalar1=float(N - NP))
        nc.vector.reciprocal(out=rinv[:, :w], in_=rrow[:, :w])
        nc.tensor.matmul(
            aux[:, 1, :w], lhsT=ones[0:1, :], rhs=rinv[0:1, :w],
            start=True, stop=True,
        )
        nc.scalar.activation(out=rbc[:, :w], in_=aux[:, 1, :w], func=COPY)

        # O = V @ A^T accumulated over m-tiles
        op = opool.tile([128, CT, 512], F32, tag="o")
        for mt in range(MT):
            st, sp_ = (mt == 0), (mt == MT - 1)
            for dt in range(CT):
                nc.tensor.matmul(
                    op[:, dt, :w],
                    lhsT=VT[:, mt, 128 * dt : 128 * (dt + 1)],
                    rhs=AT[:, mt, :w],
                    start=st,
                    stop=sp_,
                    skip_group_check=True,
                )

        # normalize all 3 d-tiles in one op: O_sb = op * bcast(1/denom)
        rv = rbc[:, :w]
        rb = bass.AP(
            tensor=rv.tensor, offset=rv.offset,
            ap=[list(rv.ap[0]), [0, CT], list(rv.ap[1])],
        )
        nc.vector.tensor_mul(out=OSB[:, :, :w], in0=op[:, :, :w], in1=rb)
        # stream this chunk of O straight to DRAM (keeps OSB tiny so V^T/A^T
        # fit in SBUF as f32)
        nc.sync.dma_start(
            out=out[:, ns].rearrange("(a p) w -> p a w", p=128),
            in_=OSB[:, :, :w],
        )


_NC = None


def make_in_maps(x, w1, w2):
    x = np.asarray(x, dtype=np.float32).reshape(N_CORES, C, N)
    xp = np.zeros((N_CORES, C, NP), np.float32)
    xp[:, :, :N] = x  # layout-only host prep: zero-pad to 25 full m-tiles
    wcat = np.ascontiguousarray(
        np.concatenate(
            [np.asarray(w1, dtype=np.float32).T, np.asarray(w2, dtype=np.float32).T],
            axis=1,
        )
    )
    return [{"xb": xp[b], "wcat": wcat} for b in range(N_CORES)]


def kernel(x: np.ndarray, w1: np.ndarray, w2: np.ndarray) -> np.ndarray:
    global _NC
    if _NC is None:
        _NC = build_bass()
    in_maps = make_in_maps(x, w1, w2)
    res = run_bass_kernel_spmd(_NC, in_maps, core_ids=list(range(N_CORES)))
    outs = np.stack([r["out"] for r in res.results])
    return outs.reshape(N_CORES, C, 56, 56)

